# revision 76
# baseline (speedup 1.0000x reference)
"""Single-head causal attention on 8 trn2 NeuronCores (Bass/Tile), fp8 edition.

Problem: x [4, 4096, 768] f32; Wk/Wq/Wv [768, 64]; out = softmax(causal(q k^T/8)) v.

Sharding (as baseline): 8 cores = 4 batches x 2 cores; per batch the 32 query
tiles split between 2 cores so both run the IDENTICAL program; slot s
(s=0..15) processes one q-tile against a key prefix of L[s] = 256*(s+1)
local keys (core h=1 gets its x permuted on host in 128-row tile pairs).

Precision plan (validated in numpy sim, rel ~1.1e-2 vs 2e-2 budget):
 - pairs p < P0=2 ("early", short-context queries: no softmax averaging to
   hide quantization noise) run fully in bf16.
 - pairs p >= P0 run in fp8e4 with DoubleRow matmuls (0.5 cyc/col):
   * projections contract c-chunk PAIRS (2x128) via DR;
   * scores put (k_hi, k_lo) in the two DR k-tiles where k_lo = k - fp8(k)
     (residual), rhs = same q tile twice via a stride-0 AP -> k-side
     quantization error cancels at zero extra PE cost;
   * AV contracts real 256-key pairs via DR.
 - all weights pre-scaled x16 (fp8e4 subnormal range); exp computes
   exp(s'/2048 - ln16) so stored weights are exp(s)/16 (fp8 max 240, exp
   overflows to Inf on TRN -> the /16 keeps max ~30); the ones-column of
   vaug is 16.0 so numerator/denominator scales cancel exactly.
 - causal masks enter as bf16 identity-matmul PSUM inits (start=True),
   mask value -122880 chosen so the Schraudolph path stays positive.
 - exp is split across engines: ACT does true exp (bias=-ln16,
   scale=1/2048); selected groups use DVE tensor_scalar Schraudolph
   (i = s'*A + B -> int32, bitcast = f32 approx exp, +-3%) with the
   fp8 convert on GPSIMD (SBUF->SBUF) to offload the ACT bottleneck.
"""

import functools
import os
import sys

import ml_dtypes
import numpy as np

if "/opt/trn_rl_repo" not in sys.path:
    sys.path.insert(0, "/opt/trn_rl_repo")

B, T, C, H = 4, 4096, 768, 64
NCHUNK = C // 128           # 6 embedding chunks
NSLOT = 16                  # q-tiles per core
NPAIR = 8                   # slot pairs / x pieces
P0 = 2                      # pairs < P0 use the bf16 path
NEG = -122880.0             # mask value (bf16-exact; Schraudolph-safe)

# Schraudolph exp: exp(s'/2048 - ln16) ~ bitcast(int32(s'*A_S + B_S));
# the bf16 variant computes the TOP 16 bits directly as int16 (bf16 bits =
# f32 bits >> 16), so the DVE tensor_scalar output is already usable as
# bf16 weights with no convert pass.
C_SCH = 369099.0
A_SCH = (2.0**23 / np.log(2.0)) / 2048.0
B_SCH = 127.0 * 2.0**23 - C_SCH - np.log(16.0) * 2.0**23 / np.log(2.0)
A_SCH16 = A_SCH / 65536.0
B_SCH16 = B_SCH / 65536.0

# local q-tile index per slot: even s -> 2s, odd s -> 2s+1
LTS = [2 * s + (s % 2) for s in range(NSLOT)]

# exp-group engine assignment: groups (p, g) in this set use the
# DVE-Schraudolph + GPSIMD-convert path instead of ACT.  Tuned against
# TimelineSim; only late pairs (p >= P0) are eligible.
_SCH = int(os.environ.get("SCH", "6"))  # every SCH-th late group on DVE
if _SCH == 5:
    # base g%4==1, but strict ACT/DVE alternation in the final pairs
    SCHRAU_GROUPS = {
        (p, g) for p in range(P0, NPAIR) for g in range(p + 1)
        if (g % 2 == 1 if p >= 6 else g % 4 == 1)
    }
elif _SCH == 6:
    SCHRAU_GROUPS = {
        (p, g) for p in range(P0, NPAIR) for g in range(p + 1)
        if (g % 2 == 1 if p == 7 else g % 4 == 1)
    }
elif _SCH < 0:
    # denser in the ACT-bound tail pairs
    SCHRAU_GROUPS = {
        (p, g) for p in range(P0, NPAIR) for g in range(p + 1)
        if g % (2 if p >= 5 else 3) == 1
    }
else:
    SCHRAU_GROUPS = {
        (p, g) for p in range(P0, NPAIR) for g in range(p + 1)
        if _SCH and g % _SCH == 1
    }
# copy-engine choices ('v' = DVE, 'a' = ACT)
ENG_OUTT = os.environ.get("ENG_OUTT", "v")
ENG_VAUG = os.environ.get("ENG_VAUG", "v")
ENG_EARLY = os.environ.get("ENG_EARLY", "a")   # kbs/vab/qbs early copies
ENG_K8 = os.environ.get("ENG_K8", "v")         # k_hi fp8 store
ENG_QT = os.environ.get("ENG_QT", "v")         # late q fp8 store
KRES = os.environ.get("KRES", "1") == "1"
AVLAG = int(os.environ.get("AVLAG", "5"))      # software-pipeline depth
WTB = int(os.environ.get("WTB", "3"))          # fp8 wt tile bufs
XPB = int(os.environ.get("XPB", "3"))          # fp8 x piece bufs
WARM = int(os.environ.get("WARM", "12"))        # PE ramp warmup matmuls
SCHAV = os.environ.get("SCHAV", "b")
ENG_VTS = os.environ.get("ENG_VTS", "a")       # vts copy: a/v/h (h=DVE for odd late pieces)
MISCB = int(os.environ.get("MISCB", "2"))      # sb_misc pool bufs           # schraudolph AV: b=bf16, 8=Pool-convert fp8 DR

# V2: all pieces get an fp8 projection (late pairs unblock early); the
# bf16 projections for pairs 0-1 run in mid-stream slack windows.
V2 = os.environ.get("V2", "0") == "1"

# emission order for pieces/attention-pairs
_EMIT = os.environ.get("EMIT", "pa")
if V2:
    EMIT_ORDER = ["p80", "p81", "p82", "at2", "p83", "at3", "pb0", "at0",
                  "p84", "at4", "pb1", "at1", "p85", "at5", "p86", "at6",
                  "p87", "at7"]
elif _EMIT == "il":
    EMIT_ORDER = ["pc0", "at0", "pc1", "at1", "pc2", "at2", "pc3", "at3",
                  "pc4", "at4", "pc5", "at5", "pc6", "at6", "pc7", "at7"]
elif _EMIT == "pa2":
    # largest pair (7) runs before pair 6 so the drain tail is shorter
    EMIT_ORDER = ["pc0", "pc1", "pc2", "at0", "pc3", "at1", "pc4", "at2",
                  "pc5", "at3", "pc6", "at4", "pc7", "at5", "at7", "at6"]
else:
    EMIT_ORDER = ["pc0", "pc1", "pc2", "at0", "pc3", "at1", "pc4", "at2",
                  "pc5", "at3", "pc6", "at4", "pc7", "at5", "at6", "at7"]

# outT PSUM slot per pair, assigned in attention emission order
_AT_SEQ = [int(s[2]) for s in EMIT_ORDER if s[:2] == "at"]
OSLOT = {pair: i % 4 for i, pair in enumerate(_AT_SEQ)}


def _build_program():
    import concourse.bass as bass
    import concourse.tile as tile
    from concourse import mybir
    from contextlib import ExitStack

    f32 = mybir.dt.float32
    f32r = mybir.dt.float32r
    bf16 = mybir.dt.bfloat16
    fp8 = mybir.dt.float8e4
    i16 = mybir.dt.int16
    EXP = mybir.ActivationFunctionType.Exp
    DR = mybir.MatmulPerfMode.DoubleRow

    nc = bass.Bass(trn_type="TRN2", target_bir_lowering=False, debug=False)

    xb = nc.dram_tensor("xb", [128, NCHUNK, P0 * 512], bf16,
                        kind="ExternalInput").ap()
    _x8cols = T if V2 else (NPAIR - P0) * 512
    x8 = nc.dram_tensor("x8", [128, NCHUNK, _x8cols], fp8,
                        kind="ExternalInput").ap()
    # packed weights: [k|v|q] along the last axis (kv fused 128 + q 64)
    wb = nc.dram_tensor("wb", [128, NCHUNK, 192], bf16,
                        kind="ExternalInput").ap()
    w8 = nc.dram_tensor("w8", [128, 3, 2, 192], fp8, kind="ExternalInput").ap()
    # mask blocks ++ identity packed: [128, 8+1, 128] bf16
    maskc = nc.dram_tensor("maskc", [128, 9, 128], bf16,
                           kind="ExternalInput").ap()
    out_d = nc.dram_tensor("out", [H + 1, NSLOT * 128], f32r,
                           kind="ExternalOutput").ap()

    with ExitStack() as ctx:
        tc = ctx.enter_context(tile.TileContext(nc))
        const = ctx.enter_context(tc.tile_pool(name="const", bufs=1))
        xp_pool = ctx.enter_context(tc.tile_pool(name="xp", bufs=1))
        wt_pool = ctx.enter_context(tc.tile_pool(name="wt", bufs=1))
        sb_misc = ctx.enter_context(tc.tile_pool(name="misc", bufs=MISCB))
        # PSUM: scores pool (3 bufs x 2 banks) doubles as per-piece
        # projection scratch; outT quad-buffered (2 banks). 16KB total.
        ps_sc = ctx.enter_context(tc.tile_pool(name="pssc", bufs=3, space="PSUM"))
        ps_oT = ctx.enter_context(tc.tile_pool(name="psot", bufs=1, space="PSUM"))

        # outT: two PSUM banks, manually quad-buffered across pairs
        # (allocated first so warmup matmuls can use it as scratch)
        outT2 = ps_oT.tile([80, 4, 256], f32, tag="oT")
        kbs = const.tile([64, P0 * 512], bf16)       # early keys bf16
        # PE pstate warmup: junk matmuls on uninitialized SBUF during the
        # initial DMA window burn through the 3us clock ramp so the real
        # projections start at 2.4GHz.  Results land in outT2 slot 0 and
        # are overwritten by the first start=True AV.
        for w in range(WARM):
            nc.tensor.matmul(outT2[:, 0, :], lhsT=kbs[:, 0:80],
                             rhs=kbs[:, 128:384], start=True, stop=True,
                             skip_group_check=True)

        # ---- constants / persistent tensors ----
        # weights first (small), then piece-0 x at chunk granularity so the
        # first projection matmuls start as soon as their chunks land
        if V2:
            w8_s = const.tile([128, 3, 2, 192], fp8)
            nc.sync.dma_start(out=w8_s, in_=w8)
            xp0 = xp_pool.tile([128, NCHUNK, 512], fp8, tag="xp8", bufs=XPB,
                               name="xp0")
            for cc in range(0, NCHUNK, 2):
                nc.sync.dma_start(out=xp0[:, cc : cc + 2, :],
                                  in_=x8[:, cc : cc + 2, 0:512])
            wb_s = const.tile([128, NCHUNK, 192], bf16)
            nc.sync.dma_start(out=wb_s, in_=wb)
        else:
            wb_s = const.tile([128, NCHUNK, 192], bf16)
            nc.sync.dma_start(out=wb_s, in_=wb)
            xp0 = xp_pool.tile([128, NCHUNK, 512], bf16, tag="xpb", bufs=2,
                               name="xp0")
            for cc in range(0, NCHUNK, 2):
                nc.sync.dma_start(out=xp0[:, cc : cc + 2, :],
                                  in_=xb[:, cc : cc + 2, 0:512])
            w8_s = const.tile([128, 3, 2, 192], fp8)
            nc.sync.dma_start(out=w8_s, in_=w8)
        maskid_s = const.tile([128, 9, 128], bf16)
        nc.sync.dma_start(out=maskid_s, in_=maskc)
        wkvb_s = wb_s[:, :, 0:128]
        wqb_s = wb_s[:, :, 128:192]
        wkv8_s = w8_s[:, :, :, 0:128]
        wq8_s = w8_s[:, :, :, 128:192]
        mask_s = maskid_s[:, 0:8, :]
        id_s = maskid_s[:, 8, :]

        kTs = const.tile([64, 2, T], fp8)            # (k_hi, k_lo) fp8
        qTs = const.tile([64, (NPAIR - P0) * 256], fp8)
        qbs = const.tile([64, P0 * 256], bf16)
        vaug8 = const.tile([128, T // 128, 80], fp8)  # v|16-col, 80-padded
        vab = const.tile([128, P0 * 4, 80], bf16)

        bias_t = const.tile([128, 1], f32)
        nc.gpsimd.memset(bias_t, float(-np.log(16.0)))
        if not KRES:
            nc.gpsimd.memset(kTs[:, 1, :], 0.0)
        # vaug ones column (=16) + zero padding cols
        nc.gpsimd.memset(vaug8[:, :, H : H + 1], 16.0)
        nc.gpsimd.memset(vaug8[:, :, H + 1 : 80], 0.0)
        nc.gpsimd.memset(vab[:, :, H : H + 1], 16.0)
        nc.gpsimd.memset(vab[:, :, H + 1 : 80], 0.0)

        # preload the exp table during the startup DMA window
        warm_in = const.tile([1, 2], f32)
        nc.gpsimd.memset(warm_in, 0.0)
        warm = sb_misc.tile([1, 2], f32r, tag="warm")
        nc.scalar.activation(warm, warm_in, EXP)

        def piece(p, mode="c"):
            # mode: "c" combined (non-V2), "8" fp8 path, "b" bf16 path
            bpath = mode == "b" or (mode == "c" and p < P0)
            early = p < P0
            if p == 0 and (mode != "b"):
                xp = xp0
            else:
                xdt = bf16 if bpath else fp8
                xp = xp_pool.tile([128, NCHUNK, 512], xdt,
                                  tag="xpb" if bpath else "xp8",
                                  bufs=2 if bpath else XPB)
                if bpath:
                    src = xb[:, :, p * 512 : (p + 1) * 512]
                elif V2:
                    src = x8[:, :, p * 512 : (p + 1) * 512]
                else:
                    src = x8[:, :, (p - P0) * 512 : (p - P0 + 1) * 512]
                # rotate x loads across DGE queues so setup latencies overlap
                dma_eng = (nc.sync, nc.gpsimd)[p % 2]
                dma_eng.dma_start(out=xp, in_=src)

            do_kb = bpath and early
            do_k8 = mode in ("c", "8")
            do_q8 = (mode == "c" and not early) or (mode == "8" and p >= P0)
            do_q = bpath or do_q8

            # ---- fused k|v projection -> scratch in the sc rotation ----
            # one sc tile [128, 4, 256] holds kv (j=0:2), q (j=2), vt (j=3)
            scr = ps_sc.tile([128, 4, 256], f32, tag="sc",
                             name=f"scr{mode}{p}")
            kv_sub = scr[:, 0:2, :]
            kv_ps = bass.AP(tensor=kv_sub.tensor, offset=kv_sub.offset,
                            ap=[kv_sub.ap[0], [1, 512]])
            if bpath:
                for c in range(NCHUNK):
                    nc.tensor.matmul(kv_ps, lhsT=wkvb_s[:, c, :],
                                     rhs=xp[:, c, :],
                                     start=(c == 0), stop=(c == NCHUNK - 1))
            else:
                for cp in range(3):
                    nc.tensor.matmul(kv_ps, lhsT=wkv8_s[:, cp, :, :],
                                     rhs=xp[:, 2 * cp : 2 * cp + 2, :],
                                     start=(cp == 0), stop=(cp == 2),
                                     perf_mode=DR)
            cp_early = (nc.scalar.copy if ENG_EARLY == "a"
                        else nc.vector.tensor_copy)
            cols = slice(p * 512, (p + 1) * 512)
            if do_kb:
                cp_early(kbs[:, cols], kv_ps[0:64, :])
            if do_k8:
                if ENG_K8 == "a":
                    nc.scalar.copy(kTs[:, 0, cols], kv_ps[0:64, :])
                else:
                    nc.vector.tensor_copy(kTs[:, 0, cols], kv_ps[0:64, :])
                if KRES:
                    nc.vector.tensor_sub(kTs[:, 1, cols], kv_ps[0:64, :],
                                         kTs[:, 0, cols])

            # ---- v: copy v^T out, transpose per 128-tile, store ----
            vts = sb_misc.tile([64, 512], bf16, tag="vts")
            _vts_dve = (ENG_VTS == "v" or
                        (ENG_VTS == "h" and p >= P0 and p % 2 == 1))
            if _vts_dve:
                nc.vector.tensor_copy(vts, kv_ps[64:128, :])
            else:
                nc.scalar.copy(vts, kv_ps[64:128, :])
            vt_sub = scr[:, 3, 0:128].bitcast(bf16)
            vt_ps = bass.AP(tensor=vt_sub.tensor, offset=vt_sub.offset,
                            ap=[vt_sub.ap[0], [64, 4], [1, 64]])
            for t in range(4):
                nc.tensor.transpose(vt_ps[:, t, :],
                                    vts[:, t * 128 : (t + 1) * 128],
                                    id_s[0:64, 0:64])
            cp_vaug = (nc.scalar.copy if ENG_VAUG == "a"
                       else nc.vector.tensor_copy)
            if do_k8:
                cp_vaug(vaug8[:, 4 * p : 4 * p + 4, 0:H], vt_ps)
            if do_kb:
                cp_early(vab[:, 4 * p : 4 * p + 4, 0:H], vt_ps)

            # ---- q projection (2-range cols: local tiles 4p, 4p+3) ----
            if not do_q:
                return
            q_ps = scr[0:64, 2, :]
            if bpath:
                for c in range(NCHUNK):
                    base = xp[:, c, 0:128]
                    rhs = bass.AP(tensor=base.tensor, offset=base.offset,
                                  ap=[base.ap[0], [384, 2], [1, 128]])
                    nc.tensor.matmul(q_ps, lhsT=wqb_s[:, c, :], rhs=rhs,
                                     start=(c == 0), stop=(c == NCHUNK - 1))
            else:
                for cp in range(3):
                    base = xp[:, 2 * cp, 0:128]
                    rhs = bass.AP(tensor=base.tensor, offset=base.offset,
                                  ap=[base.ap[0], [512, 2], [384, 2], [1, 128]])
                    nc.tensor.matmul(q_ps, lhsT=wq8_s[:, cp, :, :], rhs=rhs,
                                     start=(cp == 0), stop=(cp == 2),
                                     perf_mode=DR)
            if bpath:
                cp_early(qbs[:, p * 256 : (p + 1) * 256], q_ps)
            elif ENG_QT == "a":
                nc.scalar.copy(qTs[:, (p - P0) * 256 : (p - P0 + 1) * 256],
                               q_ps)
            else:
                nc.vector.tensor_copy(
                    qTs[:, (p - P0) * 256 : (p - P0 + 1) * 256], q_ps)

        def q_rhs(p):
            if p < P0:
                return qbs[:, p * 256 : (p + 1) * 256]
            base = qTs[:, (p - P0) * 256 : (p - P0 + 1) * 256]
            return bass.AP(tensor=base.tensor, offset=base.offset,
                           ap=[base.ap[0], [0, 2], [1, 256]])

        def emit_scores_exp(p, g):
            early = p < P0
            scg = ps_sc.tile([128, 4, 256], f32, tag="sc", name=f"sc{p}_{g}")
            masked = g == p
            if masked:
                chunks = [4 * p, 4 * p + 3, 4 * p + 1, 4 * p + 2]
                nc.tensor.matmul(scg[:, 0:2, :], lhsT=id_s,
                                 rhs=mask_s[:, 0:4, :],
                                 start=True, stop=False,
                                 skip_group_check=True)
                nc.tensor.matmul(scg[:, 2:4, :], lhsT=id_s,
                                 rhs=mask_s[:, 4:8, :],
                                 start=True, stop=False,
                                 skip_group_check=True)
            else:
                chunks = [4 * g, 4 * g + 1, 4 * g + 2, 4 * g + 3]
            for j, kc in enumerate(chunks):
                kcol = slice(kc * 128, (kc + 1) * 128)
                if early:
                    nc.tensor.matmul(scg[:, j, :], lhsT=kbs[:, kcol],
                                     rhs=q_rhs(p), start=(not masked),
                                     stop=True, skip_group_check=True)
                else:
                    nc.tensor.matmul(scg[:, j, :], lhsT=kTs[:, :, kcol],
                                     rhs=q_rhs(p), start=(not masked),
                                     stop=True, perf_mode=DR,
                                     skip_group_check=True)
            # exp
            if early:
                wt = wt_pool.tile([128, 4, 256], bf16, tag="wtb", bufs=2)
                nc.scalar.activation(wt, scg, EXP, bias=bias_t,
                                     scale=float(1.0 / 2048.0))
            elif (p, g) in SCHRAU_GROUPS:
                w16 = wt_pool.tile([128, 4, 256], i16, tag="w16", bufs=2)
                nc.vector.tensor_scalar(
                    w16, scg, float(A_SCH16), float(B_SCH16),
                    mybir.AluOpType.mult, mybir.AluOpType.add)
                if SCHAV == "8":
                    # Pool converts bf16->fp8 (SBUF->SBUF) so AV can run DR
                    wt = wt_pool.tile([128, 4, 256], fp8, tag="wt8",
                                      bufs=WTB)
                    nc.gpsimd.tensor_copy(wt, w16.bitcast(bf16))
                else:
                    wt = w16.bitcast(bf16)
            else:
                wt = wt_pool.tile([128, 4, 256], fp8, tag="wt8", bufs=WTB)
                nc.scalar.activation(wt, scg, EXP, bias=bias_t,
                                     scale=float(1.0 / 2048.0))
            return wt, chunks

        def emit_av(p, g, wt, chunks):
            early = p < P0
            outT = outT2[:, OSLOT[p], :]
            first = g == 0
            last = g == p
            if early:
                for j in range(4):
                    nc.tensor.matmul(
                        outT, lhsT=vab[:, chunks[j], :], rhs=wt[:, j, :],
                        start=(first and j == 0), stop=(last and j == 3),
                        skip_group_check=True)
            elif wt.dtype != fp8:
                # Schraudolph group: bf16 weights, mixed-dtype non-DR
                for j in range(4):
                    nc.tensor.matmul(
                        outT, lhsT=vaug8[:, chunks[j], :],
                        rhs=wt[:, j, :],
                        start=(first and j == 0), stop=(last and j == 3),
                        skip_group_check=True)
            else:
                for j in (0, 2):
                    c0, c1 = chunks[j], chunks[j + 1]
                    base = vaug8[:, c0, :]
                    lhsT = bass.AP(tensor=base.tensor, offset=base.offset,
                                   ap=[base.ap[0], [(c1 - c0) * 80, 2],
                                       [1, 80]])
                    nc.tensor.matmul(
                        outT, lhsT=lhsT, rhs=wt[:, j : j + 2, :],
                        start=(first and j == 0), stop=(last and j == 2),
                        perf_mode=DR, skip_group_check=True)
            if last:
                outT_s = sb_misc.tile([H + 1, 256], f32r, tag="oTs")
                if ENG_OUTT == "a":
                    nc.scalar.copy(outT_s, outT[0 : H + 1, :])
                else:
                    nc.vector.tensor_copy(outT_s, outT[0 : H + 1, :])
                # out-store on the Pool queue so x-piece DMAs (SP queue)
                # never wait behind it; the final store uses the idle SP
                # HWDGE queue (lower fixed overhead on the drain tail)
                _dma = nc.sync if p in _AT_SEQ[-6:] else nc.gpsimd
                _dma.dma_start(out=out_d[:, p * 256 : (p + 1) * 256],
                               in_=outT_s)

        # global software pipeline over the flat group stream: the AV of
        # group i is emitted after the scores+exp of group i+AVLAG, across
        # pair boundaries, so the exp stream never waits on a pair tail.
        from collections import deque
        pending = deque()
        for step in EMIT_ORDER:
            n = int(step[2])
            if step[:2] == "pc":
                piece(n, "c")
                continue
            if step[:2] == "p8":
                piece(n, "8")
                continue
            if step[:2] == "pb":
                piece(n, "b")
                continue
            for g in range(n + 1):
                wt, ch = emit_scores_exp(n, g)
                if len(pending) >= AVLAG:
                    emit_av(*pending.popleft())
                pending.append((n, g, wt, ch))
        while pending:
            emit_av(*pending.popleft())

    _split_matmul_waits(nc, mybir)
    return nc


def _split_matmul_waits(nc, mybir):
    """Several TRN2 instruction structs carry only ONE sync-wait slot
    (walrus: "Too many sync wait commands").  Hoist extra waits onto a
    chain of InstNoOps inserted immediately before, on the same engine —
    in-order execution preserves the semantics."""
    k = 0
    skip = {"InstAllEngineBarrier", "InstNoOp"}
    for f in nc.m.functions:
        for blk in f.blocks:
            il = blk.instructions
            i = 0
            while i < len(il):
                inst = il[i]
                if type(inst).__name__ not in skip:
                    si = inst.sync_info
                    waits = list(si.on_wait) if si is not None and si.on_wait else []
                    if len(waits) > 1:
                        for w in waits[:-1]:
                            nop = mybir.InstNoOp(
                                name=f"I-waitfix-{k}",
                                engine=inst.engine,
                                sync_info=mybir.SyncInfo(
                                    on_wait=[w], on_update=[]
                                ),
                            )
                            k += 1
                            il.insert(i, nop)
                            i += 1
                        inst.sync_info = mybir.SyncInfo(
                            on_wait=waits[-1:], on_update=list(si.on_update or [])
                        )
                i += 1


@functools.lru_cache(maxsize=1)
def _get_program():
    return _build_program()


def _host_inputs(x, Wk, Wq, Wv):
    """Build per-core input dicts."""
    f8 = ml_dtypes.float8_e4m3
    bfp = ml_dtypes.bfloat16
    diag = np.where(np.arange(128)[:, None] <= np.arange(128)[None, :],
                    0.0, NEG).astype(np.float32)
    zero = np.zeros((128, 128), np.float32)
    neg = np.full((128, 128), NEG, np.float32)

    def wlayout_b(W):
        return (W * 16).reshape(NCHUNK, 128, H).transpose(1, 0, 2).astype(bfp)

    def wlayout_8(W):
        return (W * 16).reshape(3, 2, 128, H).transpose(2, 0, 1, 3).astype(f8)

    wb_h = np.concatenate(
        [wlayout_b(Wk), wlayout_b(Wv), wlayout_b(Wq)], axis=-1)
    w8_h = np.concatenate(
        [wlayout_8(Wk), wlayout_8(Wv), wlayout_8(Wq)], axis=-1)
    ident_h = np.eye(128, dtype=np.float32)

    in_maps = []
    for core in range(8):
        b, h = core // 2, core % 2
        xt = np.asarray(x[b]).reshape(T // 128, 128, C)
        if h == 1:
            perm = np.arange(T // 128) ^ 1
            xt = xt[perm]
        # [p, c, t] = xperm[t, c*128+p]
        xT_h = xt.reshape(T, C).T.reshape(NCHUNK, 128, T).transpose(1, 0, 2)
        xb_h = xT_h[:, :, : P0 * 512].astype(bfp)
        x8_h = (xT_h if V2 else xT_h[:, :, P0 * 512 :]).astype(f8)
        # mask blocks, order: [A: j0, j1 | B: j2, j3] halves:
        #  j0 = chunk 4p  = [diag | zero]
        #  j1 = chunk 4p+3= [neg  | diag]
        #  j2 = chunk 4p+1= [X1   | zero]   X1 = neg(h=0) / zero(h=1)
        #  j3 = chunk 4p+2= [neg  | X2]     X2 = zero(h=0) / neg(h=1)
        X1 = neg if h == 0 else zero
        X2 = zero if h == 0 else neg
        maskc_h = np.stack(
            [diag, zero, neg, diag, X1, zero, neg, X2, ident_h],
            axis=1).astype(bfp)
        in_maps.append({
            "xb": xb_h, "x8": x8_h, "wb": wb_h, "w8": w8_h,
            "maskc": maskc_h,
        })
    return in_maps


def _unshard(results):
    out = np.empty((B, T, H), np.float32)
    for core in range(8):
        b, h = core // 2, core % 2
        oc = results[core]["out"]            # [H+1, NSLOT*128] unnormalized
        oc = (oc[:H] / oc[H : H + 1]).reshape(H, NSLOT, 128)
        ob = out[b].reshape(T // 128, 128, H)
        for s in range(NSLOT):
            ob[LTS[s] ^ h] = oc[:, s, :].T
    return out


def kernel(x, Wk, Wq, Wv):
    from concourse import bass_utils

    nc = _get_program()
    in_maps = _host_inputs(
        np.asarray(x, np.float32),
        np.asarray(Wk, np.float32),
        np.asarray(Wq, np.float32),
        np.asarray(Wv, np.float32),
    )
    res = bass_utils.run_bass_kernel_spmd(nc, in_maps, core_ids=list(range(8)))
    return _unshard(res.results)


# revision 80
# speedup vs baseline: 1.0084x; 1.0084x over previous
"""Single-head causal attention on 8 trn2 NeuronCores (Bass/Tile), fp8 edition.

Problem: x [4, 4096, 768] f32; Wk/Wq/Wv [768, 64]; out = softmax(causal(q k^T/8)) v.

Sharding (as baseline): 8 cores = 4 batches x 2 cores; per batch the 32 query
tiles split between 2 cores so both run the IDENTICAL program; slot s
(s=0..15) processes one q-tile against a key prefix of L[s] = 256*(s+1)
local keys (core h=1 gets its x permuted on host in 128-row tile pairs).

Precision plan (validated in numpy sim, rel ~1.1e-2 vs 2e-2 budget):
 - pairs p < P0=2 ("early", short-context queries: no softmax averaging to
   hide quantization noise) run fully in bf16.
 - pairs p >= P0 run in fp8e4 with DoubleRow matmuls (0.5 cyc/col):
   * projections contract c-chunk PAIRS (2x128) via DR;
   * scores put (k_hi, k_lo) in the two DR k-tiles where k_lo = k - fp8(k)
     (residual), rhs = same q tile twice via a stride-0 AP -> k-side
     quantization error cancels at zero extra PE cost;
   * AV contracts real 256-key pairs via DR.
 - all weights pre-scaled x16 (fp8e4 subnormal range); exp computes
   exp(s'/2048 - ln16) so stored weights are exp(s)/16 (fp8 max 240, exp
   overflows to Inf on TRN -> the /16 keeps max ~30); the ones-column of
   vaug is 16.0 so numerator/denominator scales cancel exactly.
 - causal masks enter as bf16 identity-matmul PSUM inits (start=True),
   mask value -122880 chosen so the Schraudolph path stays positive.
 - exp is split across engines: ACT does true exp (bias=-ln16,
   scale=1/2048); selected groups use DVE tensor_scalar Schraudolph
   (i = s'*A + B -> int32, bitcast = f32 approx exp, +-3%) with the
   fp8 convert on GPSIMD (SBUF->SBUF) to offload the ACT bottleneck.
"""

import functools
import os
import sys

import ml_dtypes
import numpy as np

if "/opt/trn_rl_repo" not in sys.path:
    sys.path.insert(0, "/opt/trn_rl_repo")

B, T, C, H = 4, 4096, 768, 64
NCHUNK = C // 128           # 6 embedding chunks
NSLOT = 16                  # q-tiles per core
NPAIR = 8                   # slot pairs / x pieces
P0 = int(os.environ.get("P0", "2"))  # pairs < P0 use the bf16 path
NEG = -122880.0             # mask value (bf16-exact; Schraudolph-safe)

# Schraudolph exp: exp(s'/2048 - ln16) ~ bitcast(int32(s'*A_S + B_S));
# the bf16 variant computes the TOP 16 bits directly as int16 (bf16 bits =
# f32 bits >> 16), so the DVE tensor_scalar output is already usable as
# bf16 weights with no convert pass.
C_SCH = 369099.0
A_SCH = (2.0**23 / np.log(2.0)) / 2048.0
B_SCH = 127.0 * 2.0**23 - C_SCH - np.log(16.0) * 2.0**23 / np.log(2.0)
A_SCH16 = A_SCH / 65536.0
B_SCH16 = B_SCH / 65536.0

# local q-tile index per slot: even s -> 2s, odd s -> 2s+1
LTS = [2 * s + (s % 2) for s in range(NSLOT)]

# exp-group engine assignment: groups (p, g) in this set use the
# DVE-Schraudolph + GPSIMD-convert path instead of ACT.  Tuned against
# TimelineSim; only late pairs (p >= P0) are eligible.
_SCH = int(os.environ.get("SCH", "9"))  # every SCH-th late group on DVE
if _SCH == 5:
    # base g%4==1, but strict ACT/DVE alternation in the final pairs
    SCHRAU_GROUPS = {
        (p, g) for p in range(P0, NPAIR) for g in range(p + 1)
        if (g % 2 == 1 if p >= 6 else g % 4 == 1)
    }
elif _SCH == 6:
    SCHRAU_GROUPS = {
        (p, g) for p in range(P0, NPAIR) for g in range(p + 1)
        if (g % 2 == 1 if p == 7 else g % 4 == 1)
    }
elif _SCH == 7:
    # like 6, but the final masked group (7,7) stays on ACT so DVE's
    # queue is clear for the last outT copy
    SCHRAU_GROUPS = {
        (p, g) for p in range(P0, NPAIR) for g in range(p + 1)
        if ((g % 2 == 1 and g < 7) if p == 7 else g % 4 == 1)
    }
elif _SCH == 8:
    # SCH=7 plus (7,6) on DVE to rebalance the tail
    SCHRAU_GROUPS = {(p, g) for p in range(P0, NPAIR) for g in range(p + 1)
                     if ((g in (1, 3, 5, 6)) if p == 7 else g % 4 == 1)}
elif _SCH == 9:
    SCHRAU_GROUPS = {(p, g) for p in range(P0, NPAIR) for g in range(p + 1)
                     if ((g in (1, 3, 5, 6)) if p == 7 else
                         (g in (1, 4)) if p == 6 else g % 4 == 1)}
elif _SCH == 10:
    SCHRAU_GROUPS = {(p, g) for p in range(P0, NPAIR) for g in range(p + 1)
                     if ((g in (1, 3, 4, 6)) if p == 7 else g % 4 == 1)}
elif _SCH < 0:
    # denser in the ACT-bound tail pairs
    SCHRAU_GROUPS = {
        (p, g) for p in range(P0, NPAIR) for g in range(p + 1)
        if g % (2 if p >= 5 else 3) == 1
    }
else:
    SCHRAU_GROUPS = {
        (p, g) for p in range(P0, NPAIR) for g in range(p + 1)
        if _SCH and g % _SCH == 1
    }
# copy-engine choices ('v' = DVE, 'a' = ACT)
ENG_OUTT = os.environ.get("ENG_OUTT", "v")
ENG_VAUG = os.environ.get("ENG_VAUG", "v")
ENG_EARLY = os.environ.get("ENG_EARLY", "a")   # kbs/vab/qbs early copies
ENG_K8 = os.environ.get("ENG_K8", "v")         # k_hi fp8 store
ENG_QT = os.environ.get("ENG_QT", "v")         # late q fp8 store
KRES = os.environ.get("KRES", "1") == "1"
AVLAG = int(os.environ.get("AVLAG", "5"))      # software-pipeline depth
WTB = int(os.environ.get("WTB", "3"))          # fp8 wt tile bufs
XPB = int(os.environ.get("XPB", "3"))          # fp8 x piece bufs
WARM = int(os.environ.get("WARM", "12"))        # PE ramp warmup matmuls
SCHAV = os.environ.get("SCHAV", "b")
ENG_VTS = os.environ.get("ENG_VTS", "a")       # vts copy: a/v/h (h=DVE for odd late pieces)
MISCB = int(os.environ.get("MISCB", "2"))      # sb_misc pool bufs           # schraudolph AV: b=bf16, 8=Pool-convert fp8 DR

# V2: all pieces get an fp8 projection (late pairs unblock early); the
# bf16 projections for pairs 0-1 run in mid-stream slack windows.
V2 = os.environ.get("V2", "0") == "1"

# emission order for pieces/attention-pairs
_EMIT = os.environ.get("EMIT", "pa")
if V2:
    EMIT_ORDER = ["p80", "p81", "p82", "at2", "p83", "at3", "pb0", "at0",
                  "p84", "at4", "pb1", "at1", "p85", "at5", "p86", "at6",
                  "p87", "at7"]
elif _EMIT == "il":
    EMIT_ORDER = ["pc0", "at0", "pc1", "at1", "pc2", "at2", "pc3", "at3",
                  "pc4", "at4", "pc5", "at5", "pc6", "at6", "pc7", "at7"]
elif _EMIT == "pa2":
    # largest pair (7) runs before pair 6 so the drain tail is shorter
    EMIT_ORDER = ["pc0", "pc1", "pc2", "at0", "pc3", "at1", "pc4", "at2",
                  "pc5", "at3", "pc6", "at4", "pc7", "at5", "at7", "at6"]
else:
    EMIT_ORDER = ["pc0", "pc1", "pc2", "at0", "pc3", "at1", "pc4", "at2",
                  "pc5", "at3", "pc6", "at4", "pc7", "at5", "at6", "at7"]

# outT PSUM slot per pair, assigned in attention emission order
_AT_SEQ = [int(s[2]) for s in EMIT_ORDER if s[:2] == "at"]
OSLOT = {pair: i % 4 for i, pair in enumerate(_AT_SEQ)}


def _build_program():
    import concourse.bass as bass
    import concourse.tile as tile
    from concourse import mybir
    from contextlib import ExitStack

    f32 = mybir.dt.float32
    f32r = mybir.dt.float32r
    bf16 = mybir.dt.bfloat16
    fp8 = mybir.dt.float8e4
    i16 = mybir.dt.int16
    EXP = mybir.ActivationFunctionType.Exp
    DR = mybir.MatmulPerfMode.DoubleRow

    nc = bass.Bass(trn_type="TRN2", target_bir_lowering=False, debug=False)

    xb = nc.dram_tensor("xb", [128, NCHUNK, P0 * 512], bf16,
                        kind="ExternalInput").ap()
    _x8cols = T if V2 else (NPAIR - P0) * 512
    x8 = nc.dram_tensor("x8", [128, NCHUNK, _x8cols], fp8,
                        kind="ExternalInput").ap()
    # packed weights: [k|v|q] along the last axis (kv fused 128 + q 64)
    wb = nc.dram_tensor("wb", [128, NCHUNK, 192], bf16,
                        kind="ExternalInput").ap()
    w8 = nc.dram_tensor("w8", [128, 3, 2, 192], fp8, kind="ExternalInput").ap()
    # mask blocks ++ identity packed: [128, 8+1, 128] bf16
    maskc = nc.dram_tensor("maskc", [128, 9, 128], bf16,
                           kind="ExternalInput").ap()
    out_d = nc.dram_tensor("out", [H + 1, NSLOT * 128], f32r,
                           kind="ExternalOutput").ap()

    with ExitStack() as ctx:
        tc = ctx.enter_context(tile.TileContext(nc))
        const = ctx.enter_context(tc.tile_pool(name="const", bufs=1))
        xp_pool = ctx.enter_context(tc.tile_pool(name="xp", bufs=1))
        wt_pool = ctx.enter_context(tc.tile_pool(name="wt", bufs=1))
        sb_misc = ctx.enter_context(tc.tile_pool(name="misc", bufs=MISCB))
        # PSUM: scores pool (3 bufs x 2 banks) doubles as per-piece
        # projection scratch; outT quad-buffered (2 banks). 16KB total.
        ps_sc = ctx.enter_context(tc.tile_pool(name="pssc", bufs=3, space="PSUM"))
        ps_oT = ctx.enter_context(tc.tile_pool(name="psot", bufs=1, space="PSUM"))

        # outT: two PSUM banks, manually quad-buffered across pairs
        # (allocated first so warmup matmuls can use it as scratch)
        outT2 = ps_oT.tile([80, 4, 256], f32, tag="oT")
        kbs = const.tile([64, P0 * 512], bf16)       # early keys bf16
        # PE pstate warmup: junk matmuls on uninitialized SBUF during the
        # initial DMA window burn through the 3us clock ramp so the real
        # projections start at 2.4GHz.  Results land in outT2 slot 0 and
        # are overwritten by the first start=True AV.
        for w in range(WARM):
            nc.tensor.matmul(outT2[:, 0, :], lhsT=kbs[:, 0:80],
                             rhs=kbs[:, 128:384], start=True, stop=True,
                             skip_group_check=True)

        # ---- constants / persistent tensors ----
        # weights first (small), then piece-0 x at chunk granularity so the
        # first projection matmuls start as soon as their chunks land
        if V2:
            w8_s = const.tile([128, 3, 2, 192], fp8)
            nc.sync.dma_start(out=w8_s, in_=w8)
            xp0 = xp_pool.tile([128, NCHUNK, 512], fp8, tag="xp8", bufs=XPB,
                               name="xp0")
            for cc in range(0, NCHUNK, 2):
                nc.sync.dma_start(out=xp0[:, cc : cc + 2, :],
                                  in_=x8[:, cc : cc + 2, 0:512])
            wb_s = const.tile([128, NCHUNK, 192], bf16)
            nc.sync.dma_start(out=wb_s, in_=wb)
        else:
            wb_s = const.tile([128, NCHUNK, 192], bf16)
            nc.sync.dma_start(out=wb_s, in_=wb)
            xp0 = xp_pool.tile([128, NCHUNK, 512], bf16, tag="xpb", bufs=2,
                               name="xp0")
            for cc in range(0, NCHUNK, 2):
                nc.sync.dma_start(out=xp0[:, cc : cc + 2, :],
                                  in_=xb[:, cc : cc + 2, 0:512])
            w8_s = const.tile([128, 3, 2, 192], fp8)
            nc.sync.dma_start(out=w8_s, in_=w8)
        maskid_s = const.tile([128, 9, 128], bf16)
        nc.sync.dma_start(out=maskid_s, in_=maskc)
        wkvb_s = wb_s[:, :, 0:128]
        wqb_s = wb_s[:, :, 128:192]
        wkv8_s = w8_s[:, :, :, 0:128]
        wq8_s = w8_s[:, :, :, 128:192]
        mask_s = maskid_s[:, 0:8, :]
        id_s = maskid_s[:, 8, :]

        kTs = const.tile([64, 2, T], fp8)            # (k_hi, k_lo) fp8
        qTs = const.tile([64, (NPAIR - P0) * 256], fp8)
        qbs = const.tile([64, P0 * 256], bf16)
        vaug8 = const.tile([128, T // 128, 80], fp8)  # v|16-col, 80-padded
        vab = const.tile([128, P0 * 4, 80], bf16)

        bias_t = const.tile([128, 1], f32)
        nc.gpsimd.memset(bias_t, float(-np.log(16.0)))
        if not KRES:
            nc.gpsimd.memset(kTs[:, 1, :], 0.0)
        # vaug ones column (=16) + zero padding cols
        nc.gpsimd.memset(vaug8[:, :, H : H + 1], 16.0)
        nc.gpsimd.memset(vaug8[:, :, H + 1 : 80], 0.0)
        nc.gpsimd.memset(vab[:, :, H : H + 1], 16.0)
        nc.gpsimd.memset(vab[:, :, H + 1 : 80], 0.0)

        # preload the exp table during the startup DMA window
        warm_in = const.tile([1, 2], f32)
        nc.gpsimd.memset(warm_in, 0.0)
        warm = sb_misc.tile([1, 2], f32r, tag="warm")
        nc.scalar.activation(warm, warm_in, EXP)

        def piece(p, mode="c"):
            # mode: "c" combined (non-V2), "8" fp8 path, "b" bf16 path
            bpath = mode == "b" or (mode == "c" and p < P0)
            early = p < P0
            if p == 0 and (mode != "b"):
                xp = xp0
            else:
                xdt = bf16 if bpath else fp8
                xp = xp_pool.tile([128, NCHUNK, 512], xdt,
                                  tag="xpb" if bpath else "xp8",
                                  bufs=2 if bpath else XPB)
                if bpath:
                    src = xb[:, :, p * 512 : (p + 1) * 512]
                elif V2:
                    src = x8[:, :, p * 512 : (p + 1) * 512]
                else:
                    src = x8[:, :, (p - P0) * 512 : (p - P0 + 1) * 512]
                # rotate x loads across DGE queues so setup latencies overlap
                dma_eng = (nc.sync, nc.gpsimd)[p % 2]
                dma_eng.dma_start(out=xp, in_=src)

            do_kb = bpath and early
            do_k8 = mode in ("c", "8")
            do_q8 = (mode == "c" and not early) or (mode == "8" and p >= P0)
            do_q = bpath or do_q8

            # ---- fused k|v projection -> scratch in the sc rotation ----
            # one sc tile [128, 4, 256] holds kv (j=0:2), q (j=2), vt (j=3)
            scr = ps_sc.tile([128, 4, 256], f32, tag="sc",
                             name=f"scr{mode}{p}")
            kv_sub = scr[:, 0:2, :]
            kv_ps = bass.AP(tensor=kv_sub.tensor, offset=kv_sub.offset,
                            ap=[kv_sub.ap[0], [1, 512]])
            if bpath:
                for c in range(NCHUNK):
                    nc.tensor.matmul(kv_ps, lhsT=wkvb_s[:, c, :],
                                     rhs=xp[:, c, :],
                                     start=(c == 0), stop=(c == NCHUNK - 1))
            else:
                for cp in range(3):
                    nc.tensor.matmul(kv_ps, lhsT=wkv8_s[:, cp, :, :],
                                     rhs=xp[:, 2 * cp : 2 * cp + 2, :],
                                     start=(cp == 0), stop=(cp == 2),
                                     perf_mode=DR)
            cp_early = (nc.scalar.copy if ENG_EARLY == "a"
                        else nc.vector.tensor_copy)
            cols = slice(p * 512, (p + 1) * 512)
            if do_kb:
                cp_early(kbs[:, cols], kv_ps[0:64, :])
            if do_k8:
                if ENG_K8 == "a":
                    nc.scalar.copy(kTs[:, 0, cols], kv_ps[0:64, :])
                else:
                    nc.vector.tensor_copy(kTs[:, 0, cols], kv_ps[0:64, :])
                if KRES:
                    nc.vector.tensor_sub(kTs[:, 1, cols], kv_ps[0:64, :],
                                         kTs[:, 0, cols])

            # ---- v: copy v^T out, transpose per 128-tile, store ----
            vts = sb_misc.tile([64, 512], bf16, tag="vts")
            _vts_dve = (ENG_VTS == "v" or
                        (ENG_VTS == "h" and p >= P0 and p % 2 == 1))
            if _vts_dve:
                nc.vector.tensor_copy(vts, kv_ps[64:128, :])
            else:
                nc.scalar.copy(vts, kv_ps[64:128, :])
            vt_sub = scr[:, 3, 0:128].bitcast(bf16)
            vt_ps = bass.AP(tensor=vt_sub.tensor, offset=vt_sub.offset,
                            ap=[vt_sub.ap[0], [64, 4], [1, 64]])
            for t in range(4):
                nc.tensor.transpose(vt_ps[:, t, :],
                                    vts[:, t * 128 : (t + 1) * 128],
                                    id_s[0:64, 0:64])
            cp_vaug = (nc.scalar.copy if ENG_VAUG == "a"
                       else nc.vector.tensor_copy)
            if do_k8:
                cp_vaug(vaug8[:, 4 * p : 4 * p + 4, 0:H], vt_ps)
            if do_kb:
                cp_early(vab[:, 4 * p : 4 * p + 4, 0:H], vt_ps)

            # ---- q projection (2-range cols: local tiles 4p, 4p+3) ----
            if not do_q:
                return
            q_ps = scr[0:64, 2, :]
            if bpath:
                for c in range(NCHUNK):
                    base = xp[:, c, 0:128]
                    rhs = bass.AP(tensor=base.tensor, offset=base.offset,
                                  ap=[base.ap[0], [384, 2], [1, 128]])
                    nc.tensor.matmul(q_ps, lhsT=wqb_s[:, c, :], rhs=rhs,
                                     start=(c == 0), stop=(c == NCHUNK - 1))
            else:
                for cp in range(3):
                    base = xp[:, 2 * cp, 0:128]
                    rhs = bass.AP(tensor=base.tensor, offset=base.offset,
                                  ap=[base.ap[0], [512, 2], [384, 2], [1, 128]])
                    nc.tensor.matmul(q_ps, lhsT=wq8_s[:, cp, :, :], rhs=rhs,
                                     start=(cp == 0), stop=(cp == 2),
                                     perf_mode=DR)
            if bpath:
                cp_early(qbs[:, p * 256 : (p + 1) * 256], q_ps)
            elif ENG_QT == "a":
                nc.scalar.copy(qTs[:, (p - P0) * 256 : (p - P0 + 1) * 256],
                               q_ps)
            else:
                nc.vector.tensor_copy(
                    qTs[:, (p - P0) * 256 : (p - P0 + 1) * 256], q_ps)

        def q_rhs(p):
            if p < P0:
                return qbs[:, p * 256 : (p + 1) * 256]
            base = qTs[:, (p - P0) * 256 : (p - P0 + 1) * 256]
            return bass.AP(tensor=base.tensor, offset=base.offset,
                           ap=[base.ap[0], [0, 2], [1, 256]])

        def emit_scores_exp(p, g):
            early = p < P0
            scg = ps_sc.tile([128, 4, 256], f32, tag="sc", name=f"sc{p}_{g}")
            masked = g == p
            if masked:
                chunks = [4 * p, 4 * p + 3, 4 * p + 1, 4 * p + 2]
                nc.tensor.matmul(scg[:, 0:2, :], lhsT=id_s,
                                 rhs=mask_s[:, 0:4, :],
                                 start=True, stop=False,
                                 skip_group_check=True)
                nc.tensor.matmul(scg[:, 2:4, :], lhsT=id_s,
                                 rhs=mask_s[:, 4:8, :],
                                 start=True, stop=False,
                                 skip_group_check=True)
            else:
                chunks = [4 * g, 4 * g + 1, 4 * g + 2, 4 * g + 3]
            for j, kc in enumerate(chunks):
                kcol = slice(kc * 128, (kc + 1) * 128)
                if early:
                    nc.tensor.matmul(scg[:, j, :], lhsT=kbs[:, kcol],
                                     rhs=q_rhs(p), start=(not masked),
                                     stop=True, skip_group_check=True)
                else:
                    nc.tensor.matmul(scg[:, j, :], lhsT=kTs[:, :, kcol],
                                     rhs=q_rhs(p), start=(not masked),
                                     stop=True, perf_mode=DR,
                                     skip_group_check=True)
            # exp
            if early:
                wt = wt_pool.tile([128, 4, 256], bf16, tag="wtb", bufs=2)
                nc.scalar.activation(wt, scg, EXP, bias=bias_t,
                                     scale=float(1.0 / 2048.0))
            elif (p, g) in SCHRAU_GROUPS:
                w16 = wt_pool.tile([128, 4, 256], i16, tag="w16", bufs=2)
                nc.vector.tensor_scalar(
                    w16, scg, float(A_SCH16), float(B_SCH16),
                    mybir.AluOpType.mult, mybir.AluOpType.add)
                if SCHAV == "8":
                    # Pool converts bf16->fp8 (SBUF->SBUF) so AV can run DR
                    wt = wt_pool.tile([128, 4, 256], fp8, tag="wt8",
                                      bufs=WTB)
                    nc.gpsimd.tensor_copy(wt, w16.bitcast(bf16))
                else:
                    wt = w16.bitcast(bf16)
            else:
                wt = wt_pool.tile([128, 4, 256], fp8, tag="wt8", bufs=WTB)
                nc.scalar.activation(wt, scg, EXP, bias=bias_t,
                                     scale=float(1.0 / 2048.0))
            return wt, chunks

        def emit_av(p, g, wt, chunks):
            early = p < P0
            outT = outT2[:, OSLOT[p], :]
            first = g == 0
            last = g == p
            if early:
                for j in range(4):
                    nc.tensor.matmul(
                        outT, lhsT=vab[:, chunks[j], :], rhs=wt[:, j, :],
                        start=(first and j == 0), stop=(last and j == 3),
                        skip_group_check=True)
            elif wt.dtype != fp8:
                # Schraudolph group: bf16 weights, mixed-dtype non-DR
                for j in range(4):
                    nc.tensor.matmul(
                        outT, lhsT=vaug8[:, chunks[j], :],
                        rhs=wt[:, j, :],
                        start=(first and j == 0), stop=(last and j == 3),
                        skip_group_check=True)
            else:
                for j in (0, 2):
                    c0, c1 = chunks[j], chunks[j + 1]
                    base = vaug8[:, c0, :]
                    lhsT = bass.AP(tensor=base.tensor, offset=base.offset,
                                   ap=[base.ap[0], [(c1 - c0) * 80, 2],
                                       [1, 80]])
                    nc.tensor.matmul(
                        outT, lhsT=lhsT, rhs=wt[:, j : j + 2, :],
                        start=(first and j == 0), stop=(last and j == 2),
                        perf_mode=DR, skip_group_check=True)
            if last:
                outT_s = sb_misc.tile([H + 1, 256], f32r, tag="oTs")
                if ENG_OUTT == "a":
                    nc.scalar.copy(outT_s, outT[0 : H + 1, :])
                else:
                    nc.vector.tensor_copy(outT_s, outT[0 : H + 1, :])
                # out-store on the Pool queue so x-piece DMAs (SP queue)
                # never wait behind it; the final store uses the idle SP
                # HWDGE queue (lower fixed overhead on the drain tail)
                _dma = nc.sync if p in _AT_SEQ[-6:] else nc.gpsimd
                _dma.dma_start(out=out_d[:, p * 256 : (p + 1) * 256],
                               in_=outT_s)

        # global software pipeline over the flat group stream: the AV of
        # group i is emitted after the scores+exp of group i+AVLAG, across
        # pair boundaries, so the exp stream never waits on a pair tail.
        from collections import deque
        pending = deque()
        for step in EMIT_ORDER:
            n = int(step[2])
            if step[:2] == "pc":
                piece(n, "c")
                continue
            if step[:2] == "p8":
                piece(n, "8")
                continue
            if step[:2] == "pb":
                piece(n, "b")
                continue
            for g in range(n + 1):
                wt, ch = emit_scores_exp(n, g)
                if len(pending) >= AVLAG:
                    emit_av(*pending.popleft())
                pending.append((n, g, wt, ch))
        while pending:
            emit_av(*pending.popleft())

    _split_matmul_waits(nc, mybir)
    return nc


def _split_matmul_waits(nc, mybir):
    """Several TRN2 instruction structs carry only ONE sync-wait slot
    (walrus: "Too many sync wait commands").  Hoist extra waits onto a
    chain of InstNoOps inserted immediately before, on the same engine —
    in-order execution preserves the semantics."""
    k = 0
    skip = {"InstAllEngineBarrier", "InstNoOp"}
    for f in nc.m.functions:
        for blk in f.blocks:
            il = blk.instructions
            i = 0
            while i < len(il):
                inst = il[i]
                if type(inst).__name__ not in skip:
                    si = inst.sync_info
                    waits = list(si.on_wait) if si is not None and si.on_wait else []
                    if len(waits) > 1:
                        for w in waits[:-1]:
                            nop = mybir.InstNoOp(
                                name=f"I-waitfix-{k}",
                                engine=inst.engine,
                                sync_info=mybir.SyncInfo(
                                    on_wait=[w], on_update=[]
                                ),
                            )
                            k += 1
                            il.insert(i, nop)
                            i += 1
                        inst.sync_info = mybir.SyncInfo(
                            on_wait=waits[-1:], on_update=list(si.on_update or [])
                        )
                i += 1


@functools.lru_cache(maxsize=1)
def _get_program():
    return _build_program()


def _host_inputs(x, Wk, Wq, Wv):
    """Build per-core input dicts."""
    f8 = ml_dtypes.float8_e4m3
    bfp = ml_dtypes.bfloat16
    diag = np.where(np.arange(128)[:, None] <= np.arange(128)[None, :],
                    0.0, NEG).astype(np.float32)
    zero = np.zeros((128, 128), np.float32)
    neg = np.full((128, 128), NEG, np.float32)

    def wlayout_b(W):
        return (W * 16).reshape(NCHUNK, 128, H).transpose(1, 0, 2).astype(bfp)

    def wlayout_8(W):
        return (W * 16).reshape(3, 2, 128, H).transpose(2, 0, 1, 3).astype(f8)

    wb_h = np.concatenate(
        [wlayout_b(Wk), wlayout_b(Wv), wlayout_b(Wq)], axis=-1)
    w8_h = np.concatenate(
        [wlayout_8(Wk), wlayout_8(Wv), wlayout_8(Wq)], axis=-1)
    ident_h = np.eye(128, dtype=np.float32)

    in_maps = []
    for core in range(8):
        b, h = core // 2, core % 2
        xt = np.asarray(x[b]).reshape(T // 128, 128, C)
        if h == 1:
            perm = np.arange(T // 128) ^ 1
            xt = xt[perm]
        # [p, c, t] = xperm[t, c*128+p]
        xT_h = xt.reshape(T, C).T.reshape(NCHUNK, 128, T).transpose(1, 0, 2)
        xb_h = xT_h[:, :, : P0 * 512].astype(bfp)
        x8_h = (xT_h if V2 else xT_h[:, :, P0 * 512 :]).astype(f8)
        # mask blocks, order: [A: j0, j1 | B: j2, j3] halves:
        #  j0 = chunk 4p  = [diag | zero]
        #  j1 = chunk 4p+3= [neg  | diag]
        #  j2 = chunk 4p+1= [X1   | zero]   X1 = neg(h=0) / zero(h=1)
        #  j3 = chunk 4p+2= [neg  | X2]     X2 = zero(h=0) / neg(h=1)
        X1 = neg if h == 0 else zero
        X2 = zero if h == 0 else neg
        maskc_h = np.stack(
            [diag, zero, neg, diag, X1, zero, neg, X2, ident_h],
            axis=1).astype(bfp)
        in_maps.append({
            "xb": xb_h, "x8": x8_h, "wb": wb_h, "w8": w8_h,
            "maskc": maskc_h,
        })
    return in_maps


def _unshard(results):
    out = np.empty((B, T, H), np.float32)
    for core in range(8):
        b, h = core // 2, core % 2
        oc = results[core]["out"]            # [H+1, NSLOT*128] unnormalized
        oc = (oc[:H] / oc[H : H + 1]).reshape(H, NSLOT, 128)
        ob = out[b].reshape(T // 128, 128, H)
        for s in range(NSLOT):
            ob[LTS[s] ^ h] = oc[:, s, :].T
    return out


def kernel(x, Wk, Wq, Wv):
    from concourse import bass_utils

    nc = _get_program()
    in_maps = _host_inputs(
        np.asarray(x, np.float32),
        np.asarray(Wk, np.float32),
        np.asarray(Wq, np.float32),
        np.asarray(Wv, np.float32),
    )
    res = bass_utils.run_bass_kernel_spmd(nc, in_maps, core_ids=list(range(8)))
    return _unshard(res.results)


# revision 82
# speedup vs baseline: 1.0119x; 1.0035x over previous
"""Single-head causal attention on 8 trn2 NeuronCores (Bass/Tile), fp8 edition.

Problem: x [4, 4096, 768] f32; Wk/Wq/Wv [768, 64]; out = softmax(causal(q k^T/8)) v.

Sharding (as baseline): 8 cores = 4 batches x 2 cores; per batch the 32 query
tiles split between 2 cores so both run the IDENTICAL program; slot s
(s=0..15) processes one q-tile against a key prefix of L[s] = 256*(s+1)
local keys (core h=1 gets its x permuted on host in 128-row tile pairs).

Precision plan (validated in numpy sim, rel ~1.1e-2 vs 2e-2 budget):
 - pairs p < P0=2 ("early", short-context queries: no softmax averaging to
   hide quantization noise) run fully in bf16.
 - pairs p >= P0 run in fp8e4 with DoubleRow matmuls (0.5 cyc/col):
   * projections contract c-chunk PAIRS (2x128) via DR;
   * scores put (k_hi, k_lo) in the two DR k-tiles where k_lo = k - fp8(k)
     (residual), rhs = same q tile twice via a stride-0 AP -> k-side
     quantization error cancels at zero extra PE cost;
   * AV contracts real 256-key pairs via DR.
 - all weights pre-scaled x16 (fp8e4 subnormal range); exp computes
   exp(s'/2048 - ln16) so stored weights are exp(s)/16 (fp8 max 240, exp
   overflows to Inf on TRN -> the /16 keeps max ~30); the ones-column of
   vaug is 16.0 so numerator/denominator scales cancel exactly.
 - causal masks enter as bf16 identity-matmul PSUM inits (start=True),
   mask value -122880 chosen so the Schraudolph path stays positive.
 - exp is split across engines: ACT does true exp (bias=-ln16,
   scale=1/2048); selected groups use DVE tensor_scalar Schraudolph
   (i = s'*A + B -> int32, bitcast = f32 approx exp, +-3%) with the
   fp8 convert on GPSIMD (SBUF->SBUF) to offload the ACT bottleneck.
"""

import functools
import os
import sys

import ml_dtypes
import numpy as np

if "/opt/trn_rl_repo" not in sys.path:
    sys.path.insert(0, "/opt/trn_rl_repo")

B, T, C, H = 4, 4096, 768, 64
NCHUNK = C // 128           # 6 embedding chunks
NSLOT = 16                  # q-tiles per core
NPAIR = 8                   # slot pairs / x pieces
P0 = int(os.environ.get("P0", "2"))  # pairs < P0 use the bf16 path
NEG = -122880.0             # mask value (bf16-exact; Schraudolph-safe)

# Schraudolph exp: exp(s'/2048 - ln16) ~ bitcast(int32(s'*A_S + B_S));
# the bf16 variant computes the TOP 16 bits directly as int16 (bf16 bits =
# f32 bits >> 16), so the DVE tensor_scalar output is already usable as
# bf16 weights with no convert pass.
C_SCH = 369099.0
A_SCH = (2.0**23 / np.log(2.0)) / 2048.0
B_SCH = 127.0 * 2.0**23 - C_SCH - np.log(16.0) * 2.0**23 / np.log(2.0)
A_SCH16 = A_SCH / 65536.0
B_SCH16 = B_SCH / 65536.0

# local q-tile index per slot: even s -> 2s, odd s -> 2s+1
LTS = [2 * s + (s % 2) for s in range(NSLOT)]

# exp-group engine assignment: groups (p, g) in this set use the
# DVE-Schraudolph + GPSIMD-convert path instead of ACT.  Tuned against
# TimelineSim; only late pairs (p >= P0) are eligible.
_SCH_RAW = os.environ.get("SCH", "m:94ca2112")
_LATE_GROUPS = [(p, g) for p in range(P0, NPAIR) for g in range(p + 1)]
if _SCH_RAW.startswith("m:"):
    _mask = int(_SCH_RAW[2:], 16)
    SCHRAU_GROUPS = {pg for i, pg in enumerate(_LATE_GROUPS)
                     if (_mask >> i) & 1}
    _SCH = -99
else:
    _SCH = int(_SCH_RAW)
if _SCH == -99:
    pass
elif _SCH == 5:
    # base g%4==1, but strict ACT/DVE alternation in the final pairs
    SCHRAU_GROUPS = {
        (p, g) for p in range(P0, NPAIR) for g in range(p + 1)
        if (g % 2 == 1 if p >= 6 else g % 4 == 1)
    }
elif _SCH == 6:
    SCHRAU_GROUPS = {
        (p, g) for p in range(P0, NPAIR) for g in range(p + 1)
        if (g % 2 == 1 if p == 7 else g % 4 == 1)
    }
elif _SCH == 7:
    # like 6, but the final masked group (7,7) stays on ACT so DVE's
    # queue is clear for the last outT copy
    SCHRAU_GROUPS = {
        (p, g) for p in range(P0, NPAIR) for g in range(p + 1)
        if ((g % 2 == 1 and g < 7) if p == 7 else g % 4 == 1)
    }
elif _SCH == 8:
    # SCH=7 plus (7,6) on DVE to rebalance the tail
    SCHRAU_GROUPS = {(p, g) for p in range(P0, NPAIR) for g in range(p + 1)
                     if ((g in (1, 3, 5, 6)) if p == 7 else g % 4 == 1)}
elif _SCH == 9:
    SCHRAU_GROUPS = {(p, g) for p in range(P0, NPAIR) for g in range(p + 1)
                     if ((g in (1, 3, 5, 6)) if p == 7 else
                         (g in (1, 4)) if p == 6 else g % 4 == 1)}
elif _SCH == 10:
    SCHRAU_GROUPS = {(p, g) for p in range(P0, NPAIR) for g in range(p + 1)
                     if ((g in (1, 3, 4, 6)) if p == 7 else g % 4 == 1)}
elif _SCH < 0:
    # denser in the ACT-bound tail pairs
    SCHRAU_GROUPS = {
        (p, g) for p in range(P0, NPAIR) for g in range(p + 1)
        if g % (2 if p >= 5 else 3) == 1
    }
else:
    SCHRAU_GROUPS = {
        (p, g) for p in range(P0, NPAIR) for g in range(p + 1)
        if _SCH and g % _SCH == 1
    }
# copy-engine choices ('v' = DVE, 'a' = ACT)
ENG_OUTT = os.environ.get("ENG_OUTT", "v")
ENG_VAUG = os.environ.get("ENG_VAUG", "v")
ENG_EARLY = os.environ.get("ENG_EARLY", "a")   # kbs/vab/qbs early copies
ENG_K8 = os.environ.get("ENG_K8", "v")         # k_hi fp8 store
ENG_QT = os.environ.get("ENG_QT", "v")         # late q fp8 store
KRES = os.environ.get("KRES", "1") == "1"
AVLAG = int(os.environ.get("AVLAG", "5"))      # software-pipeline depth
WTB = int(os.environ.get("WTB", "3"))          # fp8 wt tile bufs
XPB = int(os.environ.get("XPB", "3"))          # fp8 x piece bufs
WARM = int(os.environ.get("WARM", "12"))        # PE ramp warmup matmuls
SCHAV = os.environ.get("SCHAV", "b")
ENG_VTS = os.environ.get("ENG_VTS", "a")       # vts copy: a/v/h (h=DVE for odd late pieces)
MISCB = int(os.environ.get("MISCB", "2"))      # sb_misc pool bufs           # schraudolph AV: b=bf16, 8=Pool-convert fp8 DR

# V2: all pieces get an fp8 projection (late pairs unblock early); the
# bf16 projections for pairs 0-1 run in mid-stream slack windows.
V2 = os.environ.get("V2", "0") == "1"

# emission order for pieces/attention-pairs
_EMIT = os.environ.get("EMIT", "pa")
if V2:
    EMIT_ORDER = ["p80", "p81", "p82", "at2", "p83", "at3", "pb0", "at0",
                  "p84", "at4", "pb1", "at1", "p85", "at5", "p86", "at6",
                  "p87", "at7"]
elif _EMIT == "il":
    EMIT_ORDER = ["pc0", "at0", "pc1", "at1", "pc2", "at2", "pc3", "at3",
                  "pc4", "at4", "pc5", "at5", "pc6", "at6", "pc7", "at7"]
elif _EMIT == "pa2":
    # largest pair (7) runs before pair 6 so the drain tail is shorter
    EMIT_ORDER = ["pc0", "pc1", "pc2", "at0", "pc3", "at1", "pc4", "at2",
                  "pc5", "at3", "pc6", "at4", "pc7", "at5", "at7", "at6"]
else:
    EMIT_ORDER = ["pc0", "pc1", "pc2", "at0", "pc3", "at1", "pc4", "at2",
                  "pc5", "at3", "pc6", "at4", "pc7", "at5", "at6", "at7"]

# outT PSUM slot per pair, assigned in attention emission order
_AT_SEQ = [int(s[2]) for s in EMIT_ORDER if s[:2] == "at"]
OSLOT = {pair: i % 4 for i, pair in enumerate(_AT_SEQ)}


def _build_program():
    import concourse.bass as bass
    import concourse.tile as tile
    from concourse import mybir
    from contextlib import ExitStack

    f32 = mybir.dt.float32
    f32r = mybir.dt.float32r
    bf16 = mybir.dt.bfloat16
    fp8 = mybir.dt.float8e4
    i16 = mybir.dt.int16
    EXP = mybir.ActivationFunctionType.Exp
    DR = mybir.MatmulPerfMode.DoubleRow

    nc = bass.Bass(trn_type="TRN2", target_bir_lowering=False, debug=False)

    xb = nc.dram_tensor("xb", [128, NCHUNK, P0 * 512], bf16,
                        kind="ExternalInput").ap()
    _x8cols = T if V2 else (NPAIR - P0) * 512
    x8 = nc.dram_tensor("x8", [128, NCHUNK, _x8cols], fp8,
                        kind="ExternalInput").ap()
    # packed weights: [k|v|q] along the last axis (kv fused 128 + q 64)
    wb = nc.dram_tensor("wb", [128, NCHUNK, 192], bf16,
                        kind="ExternalInput").ap()
    w8 = nc.dram_tensor("w8", [128, 3, 2, 192], fp8, kind="ExternalInput").ap()
    # mask blocks ++ identity packed: [128, 8+1, 128] bf16
    maskc = nc.dram_tensor("maskc", [128, 9, 128], bf16,
                           kind="ExternalInput").ap()
    out_d = nc.dram_tensor("out", [H + 1, NSLOT * 128], f32r,
                           kind="ExternalOutput").ap()

    with ExitStack() as ctx:
        tc = ctx.enter_context(tile.TileContext(nc))
        const = ctx.enter_context(tc.tile_pool(name="const", bufs=1))
        xp_pool = ctx.enter_context(tc.tile_pool(name="xp", bufs=1))
        wt_pool = ctx.enter_context(tc.tile_pool(name="wt", bufs=1))
        sb_misc = ctx.enter_context(tc.tile_pool(name="misc", bufs=MISCB))
        # PSUM: scores pool (3 bufs x 2 banks) doubles as per-piece
        # projection scratch; outT quad-buffered (2 banks). 16KB total.
        ps_sc = ctx.enter_context(tc.tile_pool(name="pssc", bufs=3, space="PSUM"))
        ps_oT = ctx.enter_context(tc.tile_pool(name="psot", bufs=1, space="PSUM"))

        # outT: two PSUM banks, manually quad-buffered across pairs
        # (allocated first so warmup matmuls can use it as scratch)
        outT2 = ps_oT.tile([80, 4, 256], f32, tag="oT")
        kbs = const.tile([64, P0 * 512], bf16)       # early keys bf16
        # PE pstate warmup: junk matmuls on uninitialized SBUF during the
        # initial DMA window burn through the 3us clock ramp so the real
        # projections start at 2.4GHz.  Results land in outT2 slot 0 and
        # are overwritten by the first start=True AV.
        for w in range(WARM):
            nc.tensor.matmul(outT2[:, 0, :], lhsT=kbs[:, 0:80],
                             rhs=kbs[:, 128:384], start=True, stop=True,
                             skip_group_check=True)

        # ---- constants / persistent tensors ----
        # weights first (small), then piece-0 x at chunk granularity so the
        # first projection matmuls start as soon as their chunks land
        if V2:
            w8_s = const.tile([128, 3, 2, 192], fp8)
            nc.sync.dma_start(out=w8_s, in_=w8)
            xp0 = xp_pool.tile([128, NCHUNK, 512], fp8, tag="xp8", bufs=XPB,
                               name="xp0")
            for cc in range(0, NCHUNK, 2):
                nc.sync.dma_start(out=xp0[:, cc : cc + 2, :],
                                  in_=x8[:, cc : cc + 2, 0:512])
            wb_s = const.tile([128, NCHUNK, 192], bf16)
            nc.sync.dma_start(out=wb_s, in_=wb)
        else:
            wb_s = const.tile([128, NCHUNK, 192], bf16)
            nc.sync.dma_start(out=wb_s, in_=wb)
            xp0 = xp_pool.tile([128, NCHUNK, 512], bf16, tag="xpb", bufs=2,
                               name="xp0")
            for cc in range(0, NCHUNK, 2):
                nc.sync.dma_start(out=xp0[:, cc : cc + 2, :],
                                  in_=xb[:, cc : cc + 2, 0:512])
            w8_s = const.tile([128, 3, 2, 192], fp8)
            nc.sync.dma_start(out=w8_s, in_=w8)
        maskid_s = const.tile([128, 9, 128], bf16)
        nc.sync.dma_start(out=maskid_s, in_=maskc)
        wkvb_s = wb_s[:, :, 0:128]
        wqb_s = wb_s[:, :, 128:192]
        wkv8_s = w8_s[:, :, :, 0:128]
        wq8_s = w8_s[:, :, :, 128:192]
        mask_s = maskid_s[:, 0:8, :]
        id_s = maskid_s[:, 8, :]

        kTs = const.tile([64, 2, T], fp8)            # (k_hi, k_lo) fp8
        qTs = const.tile([64, (NPAIR - P0) * 256], fp8)
        qbs = const.tile([64, P0 * 256], bf16)
        vaug8 = const.tile([128, T // 128, 80], fp8)  # v|16-col, 80-padded
        vab = const.tile([128, P0 * 4, 80], bf16)

        bias_t = const.tile([128, 1], f32)
        nc.gpsimd.memset(bias_t, float(-np.log(16.0)))
        if not KRES:
            nc.gpsimd.memset(kTs[:, 1, :], 0.0)
        # vaug ones column (=16) + zero padding cols
        nc.gpsimd.memset(vaug8[:, :, H : H + 1], 16.0)
        nc.gpsimd.memset(vaug8[:, :, H + 1 : 80], 0.0)
        nc.gpsimd.memset(vab[:, :, H : H + 1], 16.0)
        nc.gpsimd.memset(vab[:, :, H + 1 : 80], 0.0)

        # preload the exp table during the startup DMA window
        warm_in = const.tile([1, 2], f32)
        nc.gpsimd.memset(warm_in, 0.0)
        warm = sb_misc.tile([1, 2], f32r, tag="warm")
        nc.scalar.activation(warm, warm_in, EXP)

        def piece(p, mode="c"):
            # mode: "c" combined (non-V2), "8" fp8 path, "b" bf16 path
            bpath = mode == "b" or (mode == "c" and p < P0)
            early = p < P0
            if p == 0 and (mode != "b"):
                xp = xp0
            else:
                xdt = bf16 if bpath else fp8
                xp = xp_pool.tile([128, NCHUNK, 512], xdt,
                                  tag="xpb" if bpath else "xp8",
                                  bufs=2 if bpath else XPB)
                if bpath:
                    src = xb[:, :, p * 512 : (p + 1) * 512]
                elif V2:
                    src = x8[:, :, p * 512 : (p + 1) * 512]
                else:
                    src = x8[:, :, (p - P0) * 512 : (p - P0 + 1) * 512]
                # rotate x loads across DGE queues so setup latencies overlap
                dma_eng = (nc.sync, nc.gpsimd)[p % 2]
                dma_eng.dma_start(out=xp, in_=src)

            do_kb = bpath and early
            do_k8 = mode in ("c", "8")
            do_q8 = (mode == "c" and not early) or (mode == "8" and p >= P0)
            do_q = bpath or do_q8

            # ---- fused k|v projection -> scratch in the sc rotation ----
            # one sc tile [128, 4, 256] holds kv (j=0:2), q (j=2), vt (j=3)
            scr = ps_sc.tile([128, 4, 256], f32, tag="sc",
                             name=f"scr{mode}{p}")
            kv_sub = scr[:, 0:2, :]
            kv_ps = bass.AP(tensor=kv_sub.tensor, offset=kv_sub.offset,
                            ap=[kv_sub.ap[0], [1, 512]])
            if bpath:
                for c in range(NCHUNK):
                    nc.tensor.matmul(kv_ps, lhsT=wkvb_s[:, c, :],
                                     rhs=xp[:, c, :],
                                     start=(c == 0), stop=(c == NCHUNK - 1))
            else:
                for cp in range(3):
                    nc.tensor.matmul(kv_ps, lhsT=wkv8_s[:, cp, :, :],
                                     rhs=xp[:, 2 * cp : 2 * cp + 2, :],
                                     start=(cp == 0), stop=(cp == 2),
                                     perf_mode=DR)
            cp_early = (nc.scalar.copy if ENG_EARLY == "a"
                        else nc.vector.tensor_copy)
            cols = slice(p * 512, (p + 1) * 512)
            if do_kb:
                cp_early(kbs[:, cols], kv_ps[0:64, :])
            if do_k8:
                if ENG_K8 == "a":
                    nc.scalar.copy(kTs[:, 0, cols], kv_ps[0:64, :])
                else:
                    nc.vector.tensor_copy(kTs[:, 0, cols], kv_ps[0:64, :])
                if KRES:
                    nc.vector.tensor_sub(kTs[:, 1, cols], kv_ps[0:64, :],
                                         kTs[:, 0, cols])

            # ---- v: copy v^T out, transpose per 128-tile, store ----
            vts = sb_misc.tile([64, 512], bf16, tag="vts")
            _vts_dve = (ENG_VTS == "v" or
                        (ENG_VTS == "h" and p >= P0 and p % 2 == 1))
            if _vts_dve:
                nc.vector.tensor_copy(vts, kv_ps[64:128, :])
            else:
                nc.scalar.copy(vts, kv_ps[64:128, :])
            vt_sub = scr[:, 3, 0:128].bitcast(bf16)
            vt_ps = bass.AP(tensor=vt_sub.tensor, offset=vt_sub.offset,
                            ap=[vt_sub.ap[0], [64, 4], [1, 64]])
            for t in range(4):
                nc.tensor.transpose(vt_ps[:, t, :],
                                    vts[:, t * 128 : (t + 1) * 128],
                                    id_s[0:64, 0:64])
            cp_vaug = (nc.scalar.copy if ENG_VAUG == "a"
                       else nc.vector.tensor_copy)
            if do_k8:
                cp_vaug(vaug8[:, 4 * p : 4 * p + 4, 0:H], vt_ps)
            if do_kb:
                cp_early(vab[:, 4 * p : 4 * p + 4, 0:H], vt_ps)

            # ---- q projection (2-range cols: local tiles 4p, 4p+3) ----
            if not do_q:
                return
            q_ps = scr[0:64, 2, :]
            if bpath:
                for c in range(NCHUNK):
                    base = xp[:, c, 0:128]
                    rhs = bass.AP(tensor=base.tensor, offset=base.offset,
                                  ap=[base.ap[0], [384, 2], [1, 128]])
                    nc.tensor.matmul(q_ps, lhsT=wqb_s[:, c, :], rhs=rhs,
                                     start=(c == 0), stop=(c == NCHUNK - 1))
            else:
                for cp in range(3):
                    base = xp[:, 2 * cp, 0:128]
                    rhs = bass.AP(tensor=base.tensor, offset=base.offset,
                                  ap=[base.ap[0], [512, 2], [384, 2], [1, 128]])
                    nc.tensor.matmul(q_ps, lhsT=wq8_s[:, cp, :, :], rhs=rhs,
                                     start=(cp == 0), stop=(cp == 2),
                                     perf_mode=DR)
            if bpath:
                cp_early(qbs[:, p * 256 : (p + 1) * 256], q_ps)
            elif ENG_QT == "a":
                nc.scalar.copy(qTs[:, (p - P0) * 256 : (p - P0 + 1) * 256],
                               q_ps)
            else:
                nc.vector.tensor_copy(
                    qTs[:, (p - P0) * 256 : (p - P0 + 1) * 256], q_ps)

        def q_rhs(p):
            if p < P0:
                return qbs[:, p * 256 : (p + 1) * 256]
            base = qTs[:, (p - P0) * 256 : (p - P0 + 1) * 256]
            return bass.AP(tensor=base.tensor, offset=base.offset,
                           ap=[base.ap[0], [0, 2], [1, 256]])

        def emit_scores_exp(p, g):
            early = p < P0
            scg = ps_sc.tile([128, 4, 256], f32, tag="sc", name=f"sc{p}_{g}")
            masked = g == p
            if masked:
                chunks = [4 * p, 4 * p + 3, 4 * p + 1, 4 * p + 2]
                nc.tensor.matmul(scg[:, 0:2, :], lhsT=id_s,
                                 rhs=mask_s[:, 0:4, :],
                                 start=True, stop=False,
                                 skip_group_check=True)
                nc.tensor.matmul(scg[:, 2:4, :], lhsT=id_s,
                                 rhs=mask_s[:, 4:8, :],
                                 start=True, stop=False,
                                 skip_group_check=True)
            else:
                chunks = [4 * g, 4 * g + 1, 4 * g + 2, 4 * g + 3]
            for j, kc in enumerate(chunks):
                kcol = slice(kc * 128, (kc + 1) * 128)
                if early:
                    nc.tensor.matmul(scg[:, j, :], lhsT=kbs[:, kcol],
                                     rhs=q_rhs(p), start=(not masked),
                                     stop=True, skip_group_check=True)
                else:
                    nc.tensor.matmul(scg[:, j, :], lhsT=kTs[:, :, kcol],
                                     rhs=q_rhs(p), start=(not masked),
                                     stop=True, perf_mode=DR,
                                     skip_group_check=True)
            # exp
            if early:
                wt = wt_pool.tile([128, 4, 256], bf16, tag="wtb", bufs=2)
                nc.scalar.activation(wt, scg, EXP, bias=bias_t,
                                     scale=float(1.0 / 2048.0))
            elif (p, g) in SCHRAU_GROUPS:
                w16 = wt_pool.tile([128, 4, 256], i16, tag="w16", bufs=2)
                nc.vector.tensor_scalar(
                    w16, scg, float(A_SCH16), float(B_SCH16),
                    mybir.AluOpType.mult, mybir.AluOpType.add)
                if SCHAV == "8":
                    # Pool converts bf16->fp8 (SBUF->SBUF) so AV can run DR
                    wt = wt_pool.tile([128, 4, 256], fp8, tag="wt8",
                                      bufs=WTB)
                    nc.gpsimd.tensor_copy(wt, w16.bitcast(bf16))
                else:
                    wt = w16.bitcast(bf16)
            else:
                wt = wt_pool.tile([128, 4, 256], fp8, tag="wt8", bufs=WTB)
                nc.scalar.activation(wt, scg, EXP, bias=bias_t,
                                     scale=float(1.0 / 2048.0))
            return wt, chunks

        def emit_av(p, g, wt, chunks):
            early = p < P0
            outT = outT2[:, OSLOT[p], :]
            first = g == 0
            last = g == p
            if early:
                for j in range(4):
                    nc.tensor.matmul(
                        outT, lhsT=vab[:, chunks[j], :], rhs=wt[:, j, :],
                        start=(first and j == 0), stop=(last and j == 3),
                        skip_group_check=True)
            elif wt.dtype != fp8:
                # Schraudolph group: bf16 weights, mixed-dtype non-DR
                for j in range(4):
                    nc.tensor.matmul(
                        outT, lhsT=vaug8[:, chunks[j], :],
                        rhs=wt[:, j, :],
                        start=(first and j == 0), stop=(last and j == 3),
                        skip_group_check=True)
            else:
                for j in (0, 2):
                    c0, c1 = chunks[j], chunks[j + 1]
                    base = vaug8[:, c0, :]
                    lhsT = bass.AP(tensor=base.tensor, offset=base.offset,
                                   ap=[base.ap[0], [(c1 - c0) * 80, 2],
                                       [1, 80]])
                    nc.tensor.matmul(
                        outT, lhsT=lhsT, rhs=wt[:, j : j + 2, :],
                        start=(first and j == 0), stop=(last and j == 2),
                        perf_mode=DR, skip_group_check=True)
            if last:
                outT_s = sb_misc.tile([H + 1, 256], f32r, tag="oTs")
                if ENG_OUTT == "a":
                    nc.scalar.copy(outT_s, outT[0 : H + 1, :])
                else:
                    nc.vector.tensor_copy(outT_s, outT[0 : H + 1, :])
                # out-store on the Pool queue so x-piece DMAs (SP queue)
                # never wait behind it; the final store uses the idle SP
                # HWDGE queue (lower fixed overhead on the drain tail)
                _dma = nc.sync if p in _AT_SEQ[-6:] else nc.gpsimd
                _dma.dma_start(out=out_d[:, p * 256 : (p + 1) * 256],
                               in_=outT_s)

        # global software pipeline over the flat group stream: the AV of
        # group i is emitted after the scores+exp of group i+AVLAG, across
        # pair boundaries, so the exp stream never waits on a pair tail.
        from collections import deque
        pending = deque()
        for step in EMIT_ORDER:
            n = int(step[2])
            if step[:2] == "pc":
                piece(n, "c")
                continue
            if step[:2] == "p8":
                piece(n, "8")
                continue
            if step[:2] == "pb":
                piece(n, "b")
                continue
            for g in range(n + 1):
                wt, ch = emit_scores_exp(n, g)
                if len(pending) >= AVLAG:
                    emit_av(*pending.popleft())
                pending.append((n, g, wt, ch))
        while pending:
            emit_av(*pending.popleft())

    _split_matmul_waits(nc, mybir)
    return nc


def _split_matmul_waits(nc, mybir):
    """Several TRN2 instruction structs carry only ONE sync-wait slot
    (walrus: "Too many sync wait commands").  Hoist extra waits onto a
    chain of InstNoOps inserted immediately before, on the same engine —
    in-order execution preserves the semantics."""
    k = 0
    skip = {"InstAllEngineBarrier", "InstNoOp"}
    for f in nc.m.functions:
        for blk in f.blocks:
            il = blk.instructions
            i = 0
            while i < len(il):
                inst = il[i]
                if type(inst).__name__ not in skip:
                    si = inst.sync_info
                    waits = list(si.on_wait) if si is not None and si.on_wait else []
                    if len(waits) > 1:
                        for w in waits[:-1]:
                            nop = mybir.InstNoOp(
                                name=f"I-waitfix-{k}",
                                engine=inst.engine,
                                sync_info=mybir.SyncInfo(
                                    on_wait=[w], on_update=[]
                                ),
                            )
                            k += 1
                            il.insert(i, nop)
                            i += 1
                        inst.sync_info = mybir.SyncInfo(
                            on_wait=waits[-1:], on_update=list(si.on_update or [])
                        )
                i += 1


@functools.lru_cache(maxsize=1)
def _get_program():
    return _build_program()


def _host_inputs(x, Wk, Wq, Wv):
    """Build per-core input dicts."""
    f8 = ml_dtypes.float8_e4m3
    bfp = ml_dtypes.bfloat16
    diag = np.where(np.arange(128)[:, None] <= np.arange(128)[None, :],
                    0.0, NEG).astype(np.float32)
    zero = np.zeros((128, 128), np.float32)
    neg = np.full((128, 128), NEG, np.float32)

    def wlayout_b(W):
        return (W * 16).reshape(NCHUNK, 128, H).transpose(1, 0, 2).astype(bfp)

    def wlayout_8(W):
        return (W * 16).reshape(3, 2, 128, H).transpose(2, 0, 1, 3).astype(f8)

    wb_h = np.concatenate(
        [wlayout_b(Wk), wlayout_b(Wv), wlayout_b(Wq)], axis=-1)
    w8_h = np.concatenate(
        [wlayout_8(Wk), wlayout_8(Wv), wlayout_8(Wq)], axis=-1)
    ident_h = np.eye(128, dtype=np.float32)

    in_maps = []
    for core in range(8):
        b, h = core // 2, core % 2
        xt = np.asarray(x[b]).reshape(T // 128, 128, C)
        if h == 1:
            perm = np.arange(T // 128) ^ 1
            xt = xt[perm]
        # [p, c, t] = xperm[t, c*128+p]
        xT_h = xt.reshape(T, C).T.reshape(NCHUNK, 128, T).transpose(1, 0, 2)
        xb_h = xT_h[:, :, : P0 * 512].astype(bfp)
        x8_h = (xT_h if V2 else xT_h[:, :, P0 * 512 :]).astype(f8)
        # mask blocks, order: [A: j0, j1 | B: j2, j3] halves:
        #  j0 = chunk 4p  = [diag | zero]
        #  j1 = chunk 4p+3= [neg  | diag]
        #  j2 = chunk 4p+1= [X1   | zero]   X1 = neg(h=0) / zero(h=1)
        #  j3 = chunk 4p+2= [neg  | X2]     X2 = zero(h=0) / neg(h=1)
        X1 = neg if h == 0 else zero
        X2 = zero if h == 0 else neg
        maskc_h = np.stack(
            [diag, zero, neg, diag, X1, zero, neg, X2, ident_h],
            axis=1).astype(bfp)
        in_maps.append({
            "xb": xb_h, "x8": x8_h, "wb": wb_h, "w8": w8_h,
            "maskc": maskc_h,
        })
    return in_maps


def _unshard(results):
    out = np.empty((B, T, H), np.float32)
    for core in range(8):
        b, h = core // 2, core % 2
        oc = results[core]["out"]            # [H+1, NSLOT*128] unnormalized
        oc = (oc[:H] / oc[H : H + 1]).reshape(H, NSLOT, 128)
        ob = out[b].reshape(T // 128, 128, H)
        for s in range(NSLOT):
            ob[LTS[s] ^ h] = oc[:, s, :].T
    return out


def kernel(x, Wk, Wq, Wv):
    from concourse import bass_utils

    nc = _get_program()
    in_maps = _host_inputs(
        np.asarray(x, np.float32),
        np.asarray(Wk, np.float32),
        np.asarray(Wq, np.float32),
        np.asarray(Wv, np.float32),
    )
    res = bass_utils.run_bass_kernel_spmd(nc, in_maps, core_ids=list(range(8)))
    return _unshard(res.results)
